# revision 5
# baseline (speedup 1.0000x reference)
"""GCN layer kernel for Trainium2 (Bass/Tile), data-parallel over batch.

Reference computation (per batch element):
    deg = A.sum(-1); d = deg ** -0.5
    t   = X @ W.T + b
    out = relu(diag(d) @ A @ diag(d) @ t)

Per-core mapping (8 cores, one batch element each):
  - A streams in as 16 natural row-tiles [128, 2048]; each is PE-transposed
    (f32r transpose-mode matmul vs identity) into a 16 MB SBUF store at_big,
    chunk (k, mu) at [:, 2048*k + 128*mu] — the tensor engine contracts over
    partitions, so A's contraction index (its column) must live on partitions.
  - Row degrees reduce on DVE from the natural tiles in the same pass;
    d = sqrt(1/deg) via DVE reciprocal + ACT sqrt (Rsqrt activation is banned).
  - t = X @ W.T via PE (X chunks PE-transposed on the fly; W.T passed
    pre-transposed from host as a layout choice); bias added from a broadcast
    tile; y = d * t scaled in place by ACT once d is known.
  - Main matmul accumulates out[mu] = sum_k AT(k,mu).T @ y[k] in PSUM f32:
    6 accumulator banks run during the stream (triangular schedule: product
    (k, mu) is runnable once row-tiles k and mu have both arrived), the
    remaining 10 row-tiles run as tail batches afterwards.
  - Drain: relu(d * psum) on ACT, then DMA out.
"""

from contextlib import ExitStack

import numpy as np

import concourse.bacc as bacc
import concourse.mybir as mybir
import concourse.tile as tile
from concourse.bass_utils import run_bass_kernel_spmd
from concourse.masks import make_identity

B = 8
N = 2048
F = 256
P = 128
NT = N // P  # 16 row tiles
FT = F // P  # 2 feature tiles
F32 = mybir.dt.float32
F32R = mybir.dt.float32r
COPY = mybir.ActivationFunctionType.Copy
RELU = mybir.ActivationFunctionType.Relu
ACC_SLOTS = 6  # PSUM banks for out accumulators (2 reserved for transposes)


def _emit(ctx: ExitStack, tc: tile.TileContext, A, X, WT, BIAS, OUT):
    nc = tc.nc

    const = ctx.enter_context(tc.tile_pool(name="const", bufs=1))
    stage = ctx.enter_context(tc.tile_pool(name="stage", bufs=3))
    xstage = ctx.enter_context(tc.tile_pool(name="xstage", bufs=2))
    xt_pool = ctx.enter_context(tc.tile_pool(name="xt", bufs=3))
    at_pool = ctx.enter_context(tc.tile_pool(name="at", bufs=1))
    outstage = ctx.enter_context(tc.tile_pool(name="outstage", bufs=4))
    psum_acc = ctx.enter_context(
        tc.tile_pool(name="psum_acc", bufs=ACC_SLOTS, space="PSUM")
    )
    psum_tr = ctx.enter_context(tc.tile_pool(name="psum_tr", bufs=2, space="PSUM"))

    ident = const.tile([P, P], F32, tag="ident")
    make_identity(nc, ident[:, :])

    # W.T resident in SBUF: two [128, 256] tiles (partition = input feature f).
    # fp32r matmul operands must be produced rounded-to-f32r, so the DMA goes
    # to an f32 staging tile and an ACT copy performs the rounding.
    wt_stage = const.tile([P, FT * F], F32, tag="wts")
    for phi in range(FT):
        nc.sync.dma_start(
            out=wt_stage[:, phi * F : (phi + 1) * F], in_=WT[phi * P : (phi + 1) * P, :]
        )
    wt_sb = const.tile([P, FT * F], F32R, tag="wt")
    nc.scalar.copy(wt_sb[:, :], wt_stage[:, :])

    # bias broadcast tile [128, 256] built via ones-column outer product
    b_row = const.tile([1, F], F32, tag="brow")
    nc.sync.dma_start(out=b_row[:, :], in_=BIAS[:, :])
    ones_row = const.tile([1, P], F32, tag="ones")
    nc.vector.memset(ones_row[:, :], 1.0)
    b_psum = psum_acc.tile([P, F], F32, tag="acc")
    nc.tensor.matmul(b_psum[:, :], ones_row[:, :], b_row[:, :], start=True, stop=True)
    b_bcast = const.tile([P, F], F32, tag="bbc")
    nc.scalar.copy(b_bcast[:, :], b_psum[:, :])

    # degree -> d = sqrt(1/deg) storage, one column per row-tile
    deg = const.tile([P, NT], F32, tag="deg")
    rec = const.tile([P, NT], F32, tag="rec")
    dinv = const.tile([P, NT], F32, tag="dinv")

    # y (first t = X W^T + b, then scaled in place by d): [128, 16*256]
    y_big = const.tile([P, NT * F], F32R, tag="y")

    # transposed adjacency store: chunk (k, mu) at [:, 2048*k + 128*mu]
    at_big = at_pool.tile([P, NT * N], F32R, tag="at")
    at_view = at_big[:, :].rearrange("p (k n) -> p k n", k=NT)

    # ---- t = X @ W.T + b (per row-tile), via PE-transposed X chunks ----
    for mu in range(NT):
        xs = xstage.tile([P, F], F32, tag="xs")
        nc.sync.dma_start(out=xs[:, :], in_=X[mu * P : (mu + 1) * P, :])
        xp = psum_tr.tile([P, F], F32, tag="tr")
        for phi in range(FT):
            nc.tensor.transpose(
                xp[:, phi * P : (phi + 1) * P],
                xs[:, phi * P : (phi + 1) * P],
                ident[:, :],
            )
        xt = xt_pool.tile([P, F], F32R, tag="xt")
        nc.scalar.copy(xt[:, :], xp[:, :])
        t_psum = psum_acc.tile([P, F], F32, tag="acc")
        for phi in range(FT):
            nc.tensor.matmul(
                t_psum[:, :],
                xt[:, phi * P : (phi + 1) * P],
                wt_sb[:, phi * F : (phi + 1) * F],
                start=(phi == 0),
                stop=(phi == FT - 1),
            )
        # t + b -> y region (DVE, PSUM read); output rounds to f32r
        with nc.allow_low_precision(reason="f32r matmul operand"):
            nc.vector.tensor_add(
                y_big[:, mu * F : (mu + 1) * F], t_psum[:, :], b_bcast[:, :]
            )

    # ---- stream A row-tiles: degree, d, y-scale, transpose, main matmul ----
    acc_tiles = {}

    def emit_product(k, mu):
        nc.tensor.matmul(
            acc_tiles[mu][:, :],
            at_view[:, k, P * mu : P * mu + P],
            y_big[:, k * F : (k + 1) * F],
            start=(k == 0),
            stop=(k == NT - 1),
        )

    def emit_drain(mu):
        os = outstage.tile([P, F], F32, tag="os")
        nc.scalar.activation(
            os[:, :], acc_tiles[mu][:, :], RELU, scale=dinv[:, mu : mu + 1]
        )
        nc.sync.dma_start(out=OUT[mu * P : (mu + 1) * P, :], in_=os[:, :])

    for i in range(NT):
        a_nat = stage.tile([P, N], F32, tag="a")
        nc.sync.dma_start(out=a_nat[:, :], in_=A[i * P : (i + 1) * P, :])
        # degree of these 128 rows; d = sqrt(1/deg)
        nc.vector.tensor_reduce(
            deg[:, i : i + 1],
            a_nat[:, :],
            axis=mybir.AxisListType.X,
            op=mybir.AluOpType.add,
        )
        nc.vector.reciprocal(rec[:, i : i + 1], deg[:, i : i + 1])
        nc.scalar.sqrt(dinv[:, i : i + 1], rec[:, i : i + 1])
        # y[i] = d[i] * t[i] (in place)
        nc.scalar.activation(
            y_big[:, i * F : (i + 1) * F],
            y_big[:, i * F : (i + 1) * F],
            COPY,
            scale=dinv[:, i : i + 1],
        )
        # transpose the 16 [128,128] chunks of this row-tile into at_big
        for g in range(4):
            tp = psum_tr.tile([P, 4 * P], F32, tag="tr")
            for j in range(4):
                nc.tensor.transpose(
                    tp[:, j * P : (j + 1) * P],
                    a_nat[:, (4 * g + j) * P : (4 * g + j + 1) * P],
                    ident[:, :],
                )
            # one strided copy: chunk (k=4g+j, mu=i) lands at at_big[:, N*k + P*i]
            nc.scalar.copy(
                at_view[:, 4 * g : 4 * g + 4, P * i : P * i + P],
                tp[:, :].rearrange("p (j r) -> p j r", j=4),
            )
        # main-matmul products that just became runnable (early accumulators):
        # every (k, mu) pair with max(k, mu) == i and mu < ACC_SLOTS
        if i < ACC_SLOTS:
            acc_tiles[i] = psum_acc.tile([P, F], F32, tag="acc", name=f"acc_{i}")
            for k in range(i + 1):
                emit_product(k, i)
        for mu in range(min(i, ACC_SLOTS)):
            emit_product(i, mu)

    # ---- drains + tail batches ----
    for mu in range(ACC_SLOTS):
        emit_drain(mu)
    for mu in range(ACC_SLOTS, NT):
        acc_tiles[mu] = psum_acc.tile([P, F], F32, tag="acc", name=f"acc_{mu}")
        for k in range(NT):
            emit_product(k, mu)
        emit_drain(mu)


_cached_nc = None


def _build():
    nc = bacc.Bacc("TRN2", target_bir_lowering=False, debug=False)
    A = nc.dram_tensor("adj", [N, N], F32, kind="ExternalInput").ap()
    X = nc.dram_tensor("x", [N, F], F32, kind="ExternalInput").ap()
    WT = nc.dram_tensor("wt", [F, F], F32, kind="ExternalInput").ap()
    BIAS = nc.dram_tensor("bias", [1, F], F32, kind="ExternalInput").ap()
    OUT = nc.dram_tensor("out", [N, F], F32, kind="ExternalOutput").ap()
    with tile.TileContext(nc) as tc:
        with ExitStack() as ctx:
            _emit(ctx, tc, A, X, WT, BIAS, OUT)
    nc.compile()
    return nc


def get_nc():
    global _cached_nc
    if _cached_nc is None:
        _cached_nc = _build()
    return _cached_nc


def make_in_maps(node_features, adj_matrix, W, b):
    node_features = np.asarray(node_features, dtype=np.float32)
    adj_matrix = np.asarray(adj_matrix, dtype=np.float32)
    wt = np.ascontiguousarray(np.asarray(W, dtype=np.float32).T)
    bias = np.ascontiguousarray(np.asarray(b, dtype=np.float32).reshape(1, F))
    return [
        {
            "adj": np.ascontiguousarray(adj_matrix[c]),
            "x": np.ascontiguousarray(node_features[c]),
            "wt": wt,
            "bias": bias,
        }
        for c in range(B)
    ]


def kernel(node_features, adj_matrix, W, b):
    nc = get_nc()
    in_maps = make_in_maps(node_features, adj_matrix, W, b)
    res = run_bass_kernel_spmd(nc, in_maps, core_ids=list(range(B)))
    return np.stack([r["out"] for r in res.results], axis=0)
